# revision 14
# baseline (speedup 1.0000x reference)
"""Trainium2 Bass kernel for nn_AttentionBlock (GroupNorm -> QKV 1x1 -> softmax
attention over 4096 tokens -> proj + residual).

Sharding: pure data-parallel over batch B=8 across the 8 NeuronCores (one
batch element per core); attention is per-batch-element so no collectives.

Per-core layout (C=512 channels, N=4096 tokens):
  - x stored [channel-part, token] as 4 chunks of [128, 4096]
  - q, k produced in fp8e4 DoubleRow pair-layout [128, 2, 4096] (dim1 =
    channel-chunk pair member); v produced directly transposed in fp8 pairs
    vT [token-part, 2, channel] (16 x [128, 2, 512])
  - logits computed transposed via DoubleRow: E^T[m, n] = sum_o k[o,m] q[o,n]
    (2 matmuls instead of 4); softmax over the partition dim m:
    exp(logit - 2.5) in fp8e4 (shift keeps exp <= ~52 < 240 e4m3 max; the
    shift cancels in normalization), denominator S via a DoubleRow
    ones-matmul broadcast across partitions
  - h_attn_unnorm[o, n] = sum_m vT[m, o] expT[m, n] (DoubleRow, fp8);
    normalization by R = 1/S applied AFTER the (linear) proj matmul, which
    keeps the reciprocal off the PE critical path
  - proj runs one n-block behind the attention m-loop so its PSUM->SBUF and
    reciprocal dependencies never stall the TensorEngine
  - QKV/proj matmuls in bf16; GroupNorm fully fp32

Self-contained: hardcodes shapes; builds + compiles the Bass graph once and
caches a persistent jitted shard_map executor over the 8 axon NeuronCores.
"""

import os
import sys

sys.path.insert(0, "/opt/trn_rl_repo")
os.environ.setdefault("MYCRO_LOCAL_CACHE", "1")

import numpy as np
import ml_dtypes

BF16 = ml_dtypes.bfloat16
FP8 = ml_dtypes.float8_e4m3

# Problem constants (hardcoded; kernel.py must not read spec/reference files)
B, C, H, W = 8, 512, 64, 64
N = H * W            # 4096 tokens
P = 128              # partitions
NCH = C // P         # 4 channel chunks
NOP = NCH // 2       # 2 channel-chunk pairs (DoubleRow)
BW = 512             # n-block width (= PSUM bank in fp32)
NB = N // BW         # 8 n-blocks
MT = N // P          # 32 m-tiles
MPAIR = MT // 2      # 16 m-tile pairs (DoubleRow)
G = 32               # groups
GS = C // G          # 16 channels per group
GPC = P // GS        # 8 groups per 128-channel chunk
EPS = 1e-6
EXP_SHIFT = 2.5      # exp(logit - shift); cancels in softmax normalization
NCORES = 8
USE_FP8 = os.environ.get("BASS_ATTN_FP8", "0") == "1"

_EXEC = None


def _build_nc():
    import concourse.bacc as bacc
    import concourse.tile as tile
    from concourse import mybir

    f32 = mybir.dt.float32
    bf16 = mybir.dt.bfloat16
    fp8 = mybir.dt.float8e4
    Alu = mybir.AluOpType
    Act = mybir.ActivationFunctionType
    DR = mybir.MatmulPerfMode.DoubleRow

    nc = bacc.Bacc(
        "TRN2", target_bir_lowering=False, debug=False, num_devices=NCORES
    )

    def din(name, shape, dt=f32):
        return nc.declare_dram_parameter(name, list(shape), dt, isOutput=False)

    x_d = din("x", [C, N])
    wq_d = din("wq", [C, C], bf16)   # [c_in, c_out], scale pre-folded
    wk_d = din("wk", [C, C], bf16)
    wv_d = din("wv", [C, C], bf16)
    wp_d = din("wp", [C, C], bf16)
    qb_d = din("qb", [C, 1])
    kb_d = din("kb", [C, 1])
    pb_d = din("pb", [C, 1])
    vbb_d = din("vbb", [P, BW])      # v bias broadcast across partitions
    gamma_d = din("gamma", [C, 1])
    beta_d = din("beta", [C, 1])
    selsum_d = din("selsum", [P, GPC])
    selbc_d = din("selbc", [GPC, P])
    ones8_d = din("ones8", [P, 2 * P], fp8)   # DoubleRow ones [P, 2, P]
    out_d = nc.declare_dram_parameter("out", [C, N], f32, isOutput=True)

    with tile.TileContext(nc) as tc:
        with (
            tc.tile_pool(name="consts", bufs=1) as consts,
            tc.tile_pool(name="qksb", bufs=1) as qkp,
            tc.tile_pool(name="vtsb", bufs=1) as vtp,
        ):
            # ---- constants / weights to SBUF ----
            def wchunks(d, tagp):
                ts = []
                for cc in range(NCH):
                    t = consts.tile([P, C], bf16, tag=f"{tagp}{cc}", name=f"{tagp}{cc}")
                    nc.gpsimd.dma_start(out=t, in_=d[cc * P : (cc + 1) * P, :])
                    ts.append(t)
                return ts

            wq_sb = wchunks(wq_d, "wq")
            wk_sb = wchunks(wk_d, "wk")
            wv_sb = wchunks(wv_d, "wv")
            wp_sb = wchunks(wp_d, "wp")

            def vchunks(d, tagp):
                ts = []
                for cc in range(NCH):
                    t = consts.tile([P, 1], f32, tag=f"{tagp}{cc}", name=f"{tagp}{cc}")
                    nc.gpsimd.dma_start(out=t, in_=d[cc * P : (cc + 1) * P, :])
                    ts.append(t)
                return ts

            qb_sb = vchunks(qb_d, "qb")
            kb_sb = vchunks(kb_d, "kb")
            pb_sb = vchunks(pb_d, "pb")
            gamma_sb = vchunks(gamma_d, "gamma")
            beta_sb = vchunks(beta_d, "beta")

            vbb_sb = consts.tile([P, BW], f32, tag="vbb")
            nc.gpsimd.dma_start(out=vbb_sb, in_=vbb_d[:, :])
            selsum_sb = consts.tile([P, GPC], f32, tag="selsum")
            nc.gpsimd.dma_start(out=selsum_sb, in_=selsum_d[:, :])
            selbc_sb = consts.tile([P, P], f32, tag="selbc")
            nc.gpsimd.dma_start(out=selbc_sb[0:GPC, :], in_=selbc_d[:, :])
            ones8_sb = consts.tile([P, 2, P], fp8, tag="ones8")
            nc.gpsimd.dma_start(
                out=ones8_sb,
                in_=ones8_d[:, :].rearrange("p (j q) -> p j q", j=2),
            )
            eps_sb = consts.tile([P, 1], f32, tag="eps")
            nc.vector.memset(eps_sb, EPS)
            negc_sb = consts.tile([P, 1], f32, tag="negc")
            nc.vector.memset(negc_sb, -EXP_SHIFT)

            # q/k in DoubleRow pair layout: [P, 2, N], dim1 = pair member j,
            # channel chunk oc = 2*op + j  (bf16 variant: flat [P, N] chunks)
            if USE_FP8:
                q_sb = [qkp.tile([P, 2, N], fp8, tag=f"q{op}", name=f"q{op}")
                        for op in range(NOP)]
                k_sb = [qkp.tile([P, 2, N], fp8, tag=f"k{op}", name=f"k{op}")
                        for op in range(NOP)]
                vt_sb = [vtp.tile([P, 2, C], fp8, tag=f"vt{t}", name=f"vt{t}")
                         for t in range(MPAIR)]
            else:
                q_sb = [qkp.tile([P, N], bf16, tag=f"q{oc}", name=f"q{oc}")
                        for oc in range(NCH)]
                k_sb = [qkp.tile([P, N], bf16, tag=f"k{oc}", name=f"k{oc}")
                        for oc in range(NCH)]
                vt_sb = [vtp.tile([P, C], bf16, tag=f"vt{t}", name=f"vt{t}")
                         for t in range(MT)]
                ones_sb = consts.tile([P, P], bf16, tag="onesb")
                nc.vector.memset(ones_sb, 1.0)

            with tc.tile_pool(name="hsb", bufs=1) as hp:
                h_sb = []
                # ---- GroupNorm (per 128-channel chunk; groups don't cross) ----
                with (
                    tc.tile_pool(name="xsb", bufs=3) as xp,
                    tc.tile_pool(name="gn", bufs=4) as gn,
                    tc.tile_pool(name="gnps", bufs=2, space="PSUM") as gnps,
                ):
                    for cc in range(NCH):
                        xt = xp.tile([P, N], f32, tag="x", name="xt")
                        nc.sync.dma_start(out=xt, in_=x_d[cc * P : (cc + 1) * P, :])
                        stats = gn.tile([P, 8, 6], f32, tag="stats")
                        for sg in range(8):
                            nc.vector.bn_stats(
                                out=stats[:, sg, :],
                                in_=xt[:, sg * 512 : (sg + 1) * 512],
                            )
                        mv = gn.tile([P, 2], f32, tag="mv")
                        nc.vector.bn_aggr(out=mv, in_=stats)
                        # rhs2 = [mean_c, E[x^2]_c]
                        rhs2 = gn.tile([P, 2], f32, tag="rhs2")
                        nc.vector.tensor_copy(out=rhs2[:, 0:1], in_=mv[:, 0:1])
                        nc.vector.scalar_tensor_tensor(
                            out=rhs2[:, 1:2], in0=mv[:, 0:1], scalar=mv[:, 0:1],
                            in1=mv[:, 1:2], op0=Alu.mult, op1=Alu.add,
                        )
                        # group sums over the 16 channels of each group
                        g_ps = gnps.tile([P, 2], f32, tag="g_ps")
                        nc.tensor.matmul(
                            out=g_ps[0:GPC, :], lhsT=selsum_sb, rhs=rhs2,
                            start=True, stop=True,
                        )
                        gs_t = gn.tile([P, 2], f32, tag="gs")
                        nc.vector.tensor_scalar(
                            out=gs_t[0:GPC, :], in0=g_ps[0:GPC, :],
                            scalar1=1.0 / GS, scalar2=None, op0=Alu.mult,
                        )
                        mean2 = gn.tile([P, 1], f32, tag="mean2")
                        nc.vector.tensor_mul(mean2[0:GPC], gs_t[0:GPC, 0:1],
                                             gs_t[0:GPC, 0:1])
                        var = gn.tile([P, 1], f32, tag="var")
                        nc.vector.tensor_sub(var[0:GPC], gs_t[0:GPC, 1:2],
                                             mean2[0:GPC])
                        sq = gn.tile([P, 1], f32, tag="sq")
                        nc.scalar.activation(
                            out=sq[0:GPC], in_=var[0:GPC], func=Act.Sqrt,
                            bias=eps_sb[0:GPC], scale=1.0,
                        )
                        gmr = gn.tile([P, 2], f32, tag="gmr")
                        nc.vector.tensor_copy(out=gmr[0:GPC, 0:1],
                                              in_=gs_t[0:GPC, 0:1])
                        nc.vector.reciprocal(out=gmr[0:GPC, 1:2], in_=sq[0:GPC])
                        # broadcast (mean_g, rstd_g) back to channels
                        bc_ps = gnps.tile([P, 2], f32, tag="bc_ps")
                        nc.tensor.matmul(
                            out=bc_ps, lhsT=selbc_sb[0:GPC, :], rhs=gmr[0:GPC, :],
                            start=True, stop=True,
                        )
                        a_t = gn.tile([P, 1], f32, tag="a")
                        nc.vector.tensor_mul(a_t, bc_ps[:, 1:2], gamma_sb[cc])
                        na_t = gn.tile([P, 1], f32, tag="na")
                        nc.vector.tensor_scalar_mul(na_t, a_t, -1.0)
                        b_t = gn.tile([P, 1], f32, tag="b")
                        nc.vector.scalar_tensor_tensor(
                            out=b_t, in0=bc_ps[:, 0:1], scalar=na_t,
                            in1=beta_sb[cc], op0=Alu.mult, op1=Alu.add,
                        )
                        # h = x*a + b on the Scalar engine (frees DVE)
                        ht = hp.tile([P, N], bf16, tag=f"h{cc}", name=f"h{cc}")
                        nc.scalar.activation(
                            out=ht, in_=xt, func=Act.Identity,
                            scale=a_t, bias=b_t,
                        )
                        h_sb.append(ht)

                # ---- QKV projections (bf16 matmuls, fp8 pair outputs) ----
                with tc.tile_pool(name="qkvps", bufs=4, space="PSUM") as qkvps:
                    for w_sb, b_sb, dst in ((wq_sb, qb_sb, q_sb),
                                            (wk_sb, kb_sb, k_sb)):
                        for oc in range(NCH):
                            for nt in range(NB):
                                nsl = slice(nt * BW, (nt + 1) * BW)
                                pt = qkvps.tile([P, BW], f32, tag="qkv")
                                for cc in range(NCH):
                                    nc.tensor.matmul(
                                        out=pt,
                                        lhsT=w_sb[cc][:, oc * P : (oc + 1) * P],
                                        rhs=h_sb[cc][:, nsl],
                                        start=(cc == 0), stop=(cc == NCH - 1),
                                    )
                                dsl = (dst[oc // 2][:, oc % 2, nsl]
                                       if USE_FP8 else dst[oc][:, nsl])
                                nc.scalar.activation(
                                    out=dsl, in_=pt, func=Act.Identity,
                                    scale=1.0, bias=b_sb[oc],
                                )
                    # vT[m, o] = sum_c h[c, m] wv[c, o]  (+ v_b along free dim)
                    for mt in range(MT):
                        msl = slice(mt * P, (mt + 1) * P)
                        pt = qkvps.tile([P, BW], f32, tag="qkv")
                        for cc in range(NCH):
                            nc.tensor.matmul(
                                out=pt, lhsT=h_sb[cc][:, msl], rhs=wv_sb[cc],
                                start=(cc == 0), stop=(cc == NCH - 1),
                            )
                        vsl = (vt_sb[mt // 2][:, mt % 2, :]
                               if USE_FP8 else vt_sb[mt])
                        nc.vector.tensor_tensor(
                            out=vsl, in0=pt, in1=vbb_sb, op=Alu.add,
                        )

            # ---- attention (fp8 DoubleRow) + delayed proj + residual ----
            with (
                tc.tile_pool(name="eps_ps", bufs=2, space="PSUM") as e_ps,
                tc.tile_pool(name="s_ps", bufs=1, space="PSUM") as s_ps,
                tc.tile_pool(name="h_ps", bufs=1, space="PSUM") as h_ps,
                tc.tile_pool(name="p_ps", bufs=1, space="PSUM") as p_ps,
                tc.tile_pool(name="expt", bufs=(18 if USE_FP8 else 34)) as expt,
                tc.tile_pool(name="epil", bufs=2) as epil,
            ):
                def emit_proj(nbp, hu, r_t):
                    nsl = slice(nbp * BW, (nbp + 1) * BW)
                    for oc2 in range(NCH):
                        pp = p_ps.tile([P, BW], f32, tag="p", name="pp")
                        for oc in range(NCH):
                            nc.tensor.matmul(
                                out=pp,
                                lhsT=wp_sb[oc][:, oc2 * P : (oc2 + 1) * P],
                                rhs=hu[oc], start=(oc == 0),
                                stop=(oc == NCH - 1),
                            )
                        xt = epil.tile([P, BW], f32, tag="xt", name="xt")
                        nc.sync.dma_start(
                            out=xt, in_=x_d[oc2 * P : (oc2 + 1) * P, nsl]
                        )
                        # out = (pp * R) + pb + x  (normalization folded here)
                        t1 = epil.tile([P, BW], f32, tag="t1", name="t1")
                        nc.vector.tensor_tensor(out=t1, in0=pp, in1=r_t,
                                                op=Alu.mult)
                        ot = epil.tile([P, BW], f32, tag="ot", name="ot")
                        nc.vector.scalar_tensor_tensor(
                            out=ot, in0=t1, scalar=pb_sb[oc2], in1=xt,
                            op0=Alu.add, op1=Alu.add,
                        )
                        nc.sync.dma_start(
                            out=out_d[oc2 * P : (oc2 + 1) * P, nsl], in_=ot
                        )

                pending = None
                for nb in range(NB):
                    nsl = slice(nb * BW, (nb + 1) * BW)
                    ps_s = s_ps.tile([P, BW], f32, tag="s", name="ps_s")
                    ph = [h_ps.tile([P, BW], f32, tag=f"h{oc}", name=f"hps{oc}")
                          for oc in range(NCH)]
                    if USE_FP8:
                        for pr in range(MPAIR):
                            et = expt.tile([P, 2, BW], fp8, tag="et", name="et")
                            for j in range(2):
                                mt = 2 * pr + j
                                msl = slice(mt * P, (mt + 1) * P)
                                pe = e_ps.tile([P, BW], f32, tag="e", name="pe")
                                for op in range(NOP):
                                    nc.tensor.matmul(
                                        out=pe, lhsT=k_sb[op][:, 0:2, msl],
                                        rhs=q_sb[op][:, 0:2, nsl],
                                        start=(op == 0), stop=(op == NOP - 1),
                                        perf_mode=DR,
                                    )
                                nc.scalar.activation(
                                    out=et[:, j, :], in_=pe, func=Act.Exp,
                                    bias=negc_sb, scale=1.0,
                                )
                            nc.tensor.matmul(
                                out=ps_s, lhsT=ones8_sb, rhs=et,
                                start=(pr == 0), stop=(pr == MPAIR - 1),
                                perf_mode=DR,
                            )
                            for oc in range(NCH):
                                nc.tensor.matmul(
                                    out=ph[oc],
                                    lhsT=vt_sb[pr][:, 0:2, oc * P : (oc + 1) * P],
                                    rhs=et,
                                    start=(pr == 0), stop=(pr == MPAIR - 1),
                                    perf_mode=DR,
                                )
                    else:
                        for mt in range(MT):
                            msl = slice(mt * P, (mt + 1) * P)
                            pe = e_ps.tile([P, BW], f32, tag="e", name="pe")
                            for oc in range(NCH):
                                nc.tensor.matmul(
                                    out=pe, lhsT=k_sb[oc][:, msl],
                                    rhs=q_sb[oc][:, nsl],
                                    start=(oc == 0), stop=(oc == NCH - 1),
                                )
                            et = expt.tile([P, BW], bf16, tag="et", name="et")
                            nc.scalar.activation(
                                out=et, in_=pe, func=Act.Exp,
                                bias=negc_sb, scale=1.0,
                            )
                            nc.tensor.matmul(
                                out=ps_s, lhsT=ones_sb, rhs=et,
                                start=(mt == 0), stop=(mt == MT - 1),
                            )
                            for oc in range(NCH):
                                nc.tensor.matmul(
                                    out=ph[oc],
                                    lhsT=vt_sb[mt][:, oc * P : (oc + 1) * P],
                                    rhs=et,
                                    start=(mt == 0), stop=(mt == MT - 1),
                                )
                    # epilogue: R = 1/S first (frees the S PSUM bank ASAP),
                    # then unnormalized h_attn -> bf16 split across ACT/DVE
                    r_t = epil.tile([P, BW], f32, tag="r", name="r_t")
                    nc.vector.reciprocal_approx_fast(out=r_t, in_=ps_s)
                    hu = []
                    for oc in range(NCH):
                        t = epil.tile([P, BW], bf16, tag=f"hu{oc}",
                                      name=f"hu{oc}")
                        if oc < 2:
                            nc.scalar.copy(out=t, in_=ph[oc])
                        else:
                            nc.vector.tensor_copy(out=t, in_=ph[oc])
                        hu.append(t)
                    if pending is not None:
                        emit_proj(*pending)
                    pending = (nb, hu, r_t)
                emit_proj(*pending)

    nc.compile()
    return nc


def _build_exec():
    import jax
    from jax.experimental.shard_map import shard_map
    from jax.sharding import Mesh, PartitionSpec

    from concourse import bass2jax, mybir

    nc = _build_nc()
    bass2jax.install_neuronx_cc_hook()

    partition_name = (
        nc.partition_id_tensor.name if nc.partition_id_tensor else None
    )
    in_names, out_names, out_avals = [], [], []
    for alloc in nc.m.functions[0].allocations:
        if not isinstance(alloc, mybir.MemoryLocationSet):
            continue
        name = alloc.memorylocations[0].name
        if alloc.kind == "ExternalInput":
            if name != partition_name:
                in_names.append(name)
        elif alloc.kind == "ExternalOutput":
            out_names.append(name)
            out_avals.append(
                jax.core.ShapedArray(
                    tuple(alloc.tensor_shape), mybir.dt.np(alloc.dtype)
                )
            )
    n_params = len(in_names)
    all_in = tuple(in_names + out_names)
    if partition_name is not None:
        all_in = all_in + (partition_name,)
    donate = tuple(range(n_params, n_params + len(out_names)))

    def _body(*args):
        operands = list(args)
        if partition_name is not None:
            operands.append(bass2jax.partition_id_tensor())
        outs = bass2jax._bass_exec_p.bind(
            *operands,
            out_avals=tuple(out_avals),
            in_names=all_in,
            out_names=tuple(out_names),
            lowering_input_output_aliases=(),
            sim_require_finite=True,
            sim_require_nnan=True,
            nc=nc,
        )
        return tuple(outs)

    devices = jax.devices()[:NCORES]
    mesh = Mesh(np.asarray(devices), ("core",))
    in_specs = (PartitionSpec("core"),) * (n_params + len(out_names))
    out_specs = (PartitionSpec("core"),) * len(out_names)
    sharded = jax.jit(
        shard_map(
            _body, mesh=mesh, in_specs=in_specs, out_specs=out_specs,
            check_rep=False,
        ),
        donate_argnums=donate,
        keep_unused=True,
    )
    return sharded, in_names, out_names, out_avals, nc


def _get_exec():
    global _EXEC
    if _EXEC is None:
        _EXEC = _build_exec()
    return _EXEC


def _selsum():
    s = np.zeros((P, GPC), np.float32)
    s[np.arange(P), np.arange(P) // GS] = 1.0
    return s


def make_concat_inputs(inputs):
    """Host-side prep: per-core shards concatenated on axis 0 (shard_map)."""
    x = np.asarray(inputs["x"], np.float32).reshape(B, C, N)
    scale = np.float32(C ** -0.5)

    def wT(w, s=None):
        w = np.asarray(w, np.float32)
        if s is not None:
            w = w * s
        return np.ascontiguousarray(w.T).astype(BF16)

    shared = {
        "wq": wT(inputs["q_w"], scale),
        "wk": wT(inputs["k_w"]),
        "wv": wT(inputs["v_w"]),
        "wp": wT(inputs["proj_w"]),
        "qb": (np.asarray(inputs["q_b"], np.float32) * scale).reshape(C, 1),
        "kb": np.asarray(inputs["k_b"], np.float32).reshape(C, 1),
        "pb": np.asarray(inputs["proj_b"], np.float32).reshape(C, 1),
        "vbb": np.ascontiguousarray(
            np.broadcast_to(
                np.asarray(inputs["v_b"], np.float32)[None, :], (P, BW)
            )
        ),
        "gamma": np.asarray(inputs["gamma"], np.float32).reshape(C, 1),
        "beta": np.asarray(inputs["beta"], np.float32).reshape(C, 1),
        "selsum": _selsum(),
        "selbc": np.ascontiguousarray(_selsum().T),
        "ones8": np.ones((P, 2 * P), FP8),
    }
    per_core = [dict(shared, x=np.ascontiguousarray(x[c]))
                for c in range(NCORES)]

    sharded, in_names, out_names, out_avals, _ = _get_exec()
    concat_in = [
        np.concatenate([per_core[c][nm] for c in range(NCORES)], axis=0)
        for nm in in_names
    ]
    return concat_in, out_avals


def run_concat(concat_in, out_avals):
    sharded = _get_exec()[0]
    concat_zeros = [
        np.zeros((NCORES * av.shape[0], *av.shape[1:]), av.dtype)
        for av in out_avals
    ]
    outs = sharded(*concat_in, *concat_zeros)
    return outs


def kernel(**inputs):
    concat_in, out_avals = make_concat_inputs(inputs)
    outs = run_concat(concat_in, out_avals)
    o = np.asarray(outs[0]).reshape(NCORES, C, N)
    return np.ascontiguousarray(o.reshape(B, C, H, W), dtype=np.float32)


# revision 15
# speedup vs baseline: 1.0025x; 1.0025x over previous
"""Trainium2 Bass kernel for nn_AttentionBlock (GroupNorm -> QKV 1x1 -> softmax
attention over 4096 tokens -> proj + residual).

Sharding: pure data-parallel over batch B=8 across the 8 NeuronCores (one
batch element per core); attention is per-batch-element so no collectives.

Per-core layout (C=512 channels, N=4096 tokens):
  - x stored [channel-part, token] as 4 chunks of [128, 4096]
  - q, k produced in fp8e4 DoubleRow pair-layout [128, 2, 4096] (dim1 =
    channel-chunk pair member); v produced directly transposed in fp8 pairs
    vT [token-part, 2, channel] (16 x [128, 2, 512])
  - logits computed transposed via DoubleRow: E^T[m, n] = sum_o k[o,m] q[o,n]
    (2 matmuls instead of 4); softmax over the partition dim m:
    exp(logit - 2.5) in fp8e4 (shift keeps exp <= ~52 < 240 e4m3 max; the
    shift cancels in normalization), denominator S via a DoubleRow
    ones-matmul broadcast across partitions
  - h_attn_unnorm[o, n] = sum_m vT[m, o] expT[m, n] (DoubleRow, fp8);
    normalization by R = 1/S applied AFTER the (linear) proj matmul, which
    keeps the reciprocal off the PE critical path
  - proj runs one n-block behind the attention m-loop so its PSUM->SBUF and
    reciprocal dependencies never stall the TensorEngine
  - QKV/proj matmuls in bf16; GroupNorm fully fp32

Self-contained: hardcodes shapes; builds + compiles the Bass graph once and
caches a persistent jitted shard_map executor over the 8 axon NeuronCores.
"""

import os
import sys

sys.path.insert(0, "/opt/trn_rl_repo")
os.environ.setdefault("MYCRO_LOCAL_CACHE", "1")

import numpy as np
import ml_dtypes

BF16 = ml_dtypes.bfloat16
FP8 = ml_dtypes.float8_e4m3

# Problem constants (hardcoded; kernel.py must not read spec/reference files)
B, C, H, W = 8, 512, 64, 64
N = H * W            # 4096 tokens
P = 128              # partitions
NCH = C // P         # 4 channel chunks
NOP = NCH // 2       # 2 channel-chunk pairs (DoubleRow)
BW = 512             # n-block width (= PSUM bank in fp32)
NB = N // BW         # 8 n-blocks
MT = N // P          # 32 m-tiles
MPAIR = MT // 2      # 16 m-tile pairs (DoubleRow)
G = 32               # groups
GS = C // G          # 16 channels per group
GPC = P // GS        # 8 groups per 128-channel chunk
EPS = 1e-6
EXP_SHIFT = 2.5      # exp(logit - shift); cancels in softmax normalization
NCORES = 8
USE_FP8 = os.environ.get("BASS_ATTN_FP8", "0") == "1"

_EXEC = None


def _build_nc():
    import concourse.bacc as bacc
    import concourse.tile as tile
    from concourse import mybir

    f32 = mybir.dt.float32
    bf16 = mybir.dt.bfloat16
    fp8 = mybir.dt.float8e4
    Alu = mybir.AluOpType
    Act = mybir.ActivationFunctionType
    DR = mybir.MatmulPerfMode.DoubleRow

    nc = bacc.Bacc(
        "TRN2", target_bir_lowering=False, debug=False, num_devices=NCORES
    )

    def din(name, shape, dt=f32):
        return nc.declare_dram_parameter(name, list(shape), dt, isOutput=False)

    x_d = din("x", [C, N])
    wq_d = din("wq", [C, C], bf16)   # [c_in, c_out], scale pre-folded
    wk_d = din("wk", [C, C], bf16)
    wv_d = din("wv", [C, C], bf16)
    wp_d = din("wp", [C, C], bf16)
    qb_d = din("qb", [C, 1])
    kb_d = din("kb", [C, 1])
    pb_d = din("pb", [C, 1])
    vbb_d = din("vbb", [P, BW])      # v bias broadcast across partitions
    gamma_d = din("gamma", [C, 1])
    beta_d = din("beta", [C, 1])
    selsum_d = din("selsum", [P, GPC])
    selbc_d = din("selbc", [GPC, P])
    ones8_d = din("ones8", [P, 2 * P], fp8)   # DoubleRow ones [P, 2, P]
    out_d = nc.declare_dram_parameter("out", [C, N], f32, isOutput=True)

    with tile.TileContext(nc) as tc:
        with (
            tc.tile_pool(name="consts", bufs=1) as consts,
            tc.tile_pool(name="qksb", bufs=1) as qkp,
            tc.tile_pool(name="vtsb", bufs=1) as vtp,
        ):
            # ---- constants / weights to SBUF ----
            def wchunks(d, tagp):
                ts = []
                for cc in range(NCH):
                    t = consts.tile([P, C], bf16, tag=f"{tagp}{cc}", name=f"{tagp}{cc}")
                    nc.gpsimd.dma_start(out=t, in_=d[cc * P : (cc + 1) * P, :])
                    ts.append(t)
                return ts

            wq_sb = wchunks(wq_d, "wq")
            wk_sb = wchunks(wk_d, "wk")
            wv_sb = wchunks(wv_d, "wv")
            wp_sb = wchunks(wp_d, "wp")

            def vchunks(d, tagp):
                ts = []
                for cc in range(NCH):
                    t = consts.tile([P, 1], f32, tag=f"{tagp}{cc}", name=f"{tagp}{cc}")
                    nc.gpsimd.dma_start(out=t, in_=d[cc * P : (cc + 1) * P, :])
                    ts.append(t)
                return ts

            qb_sb = vchunks(qb_d, "qb")
            kb_sb = vchunks(kb_d, "kb")
            pb_sb = vchunks(pb_d, "pb")
            gamma_sb = vchunks(gamma_d, "gamma")
            beta_sb = vchunks(beta_d, "beta")

            vbb_sb = consts.tile([P, BW], f32, tag="vbb")
            nc.gpsimd.dma_start(out=vbb_sb, in_=vbb_d[:, :])
            selsum_sb = consts.tile([P, GPC], f32, tag="selsum")
            nc.gpsimd.dma_start(out=selsum_sb, in_=selsum_d[:, :])
            selbc_sb = consts.tile([P, P], f32, tag="selbc")
            nc.gpsimd.dma_start(out=selbc_sb[0:GPC, :], in_=selbc_d[:, :])
            ones8_sb = consts.tile([P, 2, P], fp8, tag="ones8")
            nc.gpsimd.dma_start(
                out=ones8_sb,
                in_=ones8_d[:, :].rearrange("p (j q) -> p j q", j=2),
            )
            eps_sb = consts.tile([P, 1], f32, tag="eps")
            nc.vector.memset(eps_sb, EPS)
            negc_sb = consts.tile([P, 1], f32, tag="negc")
            nc.vector.memset(negc_sb, -EXP_SHIFT)

            # q/k in DoubleRow pair layout: [P, 2, N], dim1 = pair member j,
            # channel chunk oc = 2*op + j  (bf16 variant: flat [P, N] chunks)
            if USE_FP8:
                q_sb = [qkp.tile([P, 2, N], fp8, tag=f"q{op}", name=f"q{op}")
                        for op in range(NOP)]
                k_sb = [qkp.tile([P, 2, N], fp8, tag=f"k{op}", name=f"k{op}")
                        for op in range(NOP)]
                vt_sb = [vtp.tile([P, 2, C], fp8, tag=f"vt{t}", name=f"vt{t}")
                         for t in range(MPAIR)]
            else:
                q_sb = [qkp.tile([P, N], bf16, tag=f"q{oc}", name=f"q{oc}")
                        for oc in range(NCH)]
                k_sb = [qkp.tile([P, N], bf16, tag=f"k{oc}", name=f"k{oc}")
                        for oc in range(NCH)]
                vt_sb = [vtp.tile([P, C], bf16, tag=f"vt{t}", name=f"vt{t}")
                         for t in range(MT)]
                ones_sb = consts.tile([P, P], bf16, tag="onesb")
                nc.vector.memset(ones_sb, 1.0)

            with tc.tile_pool(name="hsb", bufs=1) as hp:
                h_sb = []
                # ---- GroupNorm (per 128-channel chunk; groups don't cross) ----
                with (
                    tc.tile_pool(name="xsb", bufs=2) as xp,
                    tc.tile_pool(name="gn", bufs=2) as gn,
                    tc.tile_pool(name="gnps", bufs=2, space="PSUM") as gnps,
                ):
                    for cc in range(NCH):
                        xt = xp.tile([P, N], f32, tag="x", name="xt")
                        nc.sync.dma_start(out=xt, in_=x_d[cc * P : (cc + 1) * P, :])
                        stats = gn.tile([P, 8, 6], f32, tag="stats")
                        for sg in range(8):
                            nc.vector.bn_stats(
                                out=stats[:, sg, :],
                                in_=xt[:, sg * 512 : (sg + 1) * 512],
                            )
                        mv = gn.tile([P, 2], f32, tag="mv")
                        nc.vector.bn_aggr(out=mv, in_=stats)
                        # rhs2 = [mean_c, E[x^2]_c]
                        rhs2 = gn.tile([P, 2], f32, tag="rhs2")
                        nc.vector.tensor_copy(out=rhs2[:, 0:1], in_=mv[:, 0:1])
                        nc.vector.scalar_tensor_tensor(
                            out=rhs2[:, 1:2], in0=mv[:, 0:1], scalar=mv[:, 0:1],
                            in1=mv[:, 1:2], op0=Alu.mult, op1=Alu.add,
                        )
                        # group sums over the 16 channels of each group
                        g_ps = gnps.tile([P, 2], f32, tag="g_ps")
                        nc.tensor.matmul(
                            out=g_ps[0:GPC, :], lhsT=selsum_sb, rhs=rhs2,
                            start=True, stop=True,
                        )
                        gs_t = gn.tile([P, 2], f32, tag="gs")
                        nc.vector.tensor_scalar(
                            out=gs_t[0:GPC, :], in0=g_ps[0:GPC, :],
                            scalar1=1.0 / GS, scalar2=None, op0=Alu.mult,
                        )
                        mean2 = gn.tile([P, 1], f32, tag="mean2")
                        nc.vector.tensor_mul(mean2[0:GPC], gs_t[0:GPC, 0:1],
                                             gs_t[0:GPC, 0:1])
                        var = gn.tile([P, 1], f32, tag="var")
                        nc.vector.tensor_sub(var[0:GPC], gs_t[0:GPC, 1:2],
                                             mean2[0:GPC])
                        sq = gn.tile([P, 1], f32, tag="sq")
                        nc.scalar.activation(
                            out=sq[0:GPC], in_=var[0:GPC], func=Act.Sqrt,
                            bias=eps_sb[0:GPC], scale=1.0,
                        )
                        gmr = gn.tile([P, 2], f32, tag="gmr")
                        nc.vector.tensor_copy(out=gmr[0:GPC, 0:1],
                                              in_=gs_t[0:GPC, 0:1])
                        nc.vector.reciprocal(out=gmr[0:GPC, 1:2], in_=sq[0:GPC])
                        # broadcast (mean_g, rstd_g) back to channels
                        bc_ps = gnps.tile([P, 2], f32, tag="bc_ps")
                        nc.tensor.matmul(
                            out=bc_ps, lhsT=selbc_sb[0:GPC, :], rhs=gmr[0:GPC, :],
                            start=True, stop=True,
                        )
                        a_t = gn.tile([P, 1], f32, tag="a")
                        nc.vector.tensor_mul(a_t, bc_ps[:, 1:2], gamma_sb[cc])
                        na_t = gn.tile([P, 1], f32, tag="na")
                        nc.vector.tensor_scalar_mul(na_t, a_t, -1.0)
                        b_t = gn.tile([P, 1], f32, tag="b")
                        nc.vector.scalar_tensor_tensor(
                            out=b_t, in0=bc_ps[:, 0:1], scalar=na_t,
                            in1=beta_sb[cc], op0=Alu.mult, op1=Alu.add,
                        )
                        # h = x*a + b on the Scalar engine (frees DVE)
                        ht = hp.tile([P, N], bf16, tag=f"h{cc}", name=f"h{cc}")
                        nc.scalar.activation(
                            out=ht, in_=xt, func=Act.Identity,
                            scale=a_t, bias=b_t,
                        )
                        h_sb.append(ht)

                # ---- QKV projections (bf16 matmuls, fp8 pair outputs) ----
                with tc.tile_pool(name="qkvps", bufs=4, space="PSUM") as qkvps:
                    for w_sb, b_sb, dst in ((wq_sb, qb_sb, q_sb),
                                            (wk_sb, kb_sb, k_sb)):
                        for oc in range(NCH):
                            for nt in range(NB):
                                nsl = slice(nt * BW, (nt + 1) * BW)
                                pt = qkvps.tile([P, BW], f32, tag="qkv")
                                for cc in range(NCH):
                                    nc.tensor.matmul(
                                        out=pt,
                                        lhsT=w_sb[cc][:, oc * P : (oc + 1) * P],
                                        rhs=h_sb[cc][:, nsl],
                                        start=(cc == 0), stop=(cc == NCH - 1),
                                    )
                                dsl = (dst[oc // 2][:, oc % 2, nsl]
                                       if USE_FP8 else dst[oc][:, nsl])
                                nc.scalar.activation(
                                    out=dsl, in_=pt, func=Act.Identity,
                                    scale=1.0, bias=b_sb[oc],
                                )
                    # vT[m, o] = sum_c h[c, m] wv[c, o]  (+ v_b along free dim)
                    for mt in range(MT):
                        msl = slice(mt * P, (mt + 1) * P)
                        pt = qkvps.tile([P, BW], f32, tag="qkv")
                        for cc in range(NCH):
                            nc.tensor.matmul(
                                out=pt, lhsT=h_sb[cc][:, msl], rhs=wv_sb[cc],
                                start=(cc == 0), stop=(cc == NCH - 1),
                            )
                        vsl = (vt_sb[mt // 2][:, mt % 2, :]
                               if USE_FP8 else vt_sb[mt])
                        nc.vector.tensor_tensor(
                            out=vsl, in0=pt, in1=vbb_sb, op=Alu.add,
                        )

            # ---- attention (fp8 DoubleRow) + delayed proj + residual ----
            with (
                tc.tile_pool(name="eps_ps", bufs=2, space="PSUM") as e_ps,
                tc.tile_pool(name="s_ps", bufs=1, space="PSUM") as s_ps,
                tc.tile_pool(name="h_ps", bufs=1, space="PSUM") as h_ps,
                tc.tile_pool(name="p_ps", bufs=1, space="PSUM") as p_ps,
                tc.tile_pool(name="expt", bufs=(18 if USE_FP8 else 34)) as expt,
                tc.tile_pool(name="epil", bufs=2) as epil,
            ):
                def emit_proj(nbp, hu, r_t):
                    nsl = slice(nbp * BW, (nbp + 1) * BW)
                    for oc2 in range(NCH):
                        pp = p_ps.tile([P, BW], f32, tag="p", name="pp")
                        for oc in range(NCH):
                            nc.tensor.matmul(
                                out=pp,
                                lhsT=wp_sb[oc][:, oc2 * P : (oc2 + 1) * P],
                                rhs=hu[oc], start=(oc == 0),
                                stop=(oc == NCH - 1),
                            )
                        xt = epil.tile([P, BW], f32, tag="xt", name="xt")
                        nc.sync.dma_start(
                            out=xt, in_=x_d[oc2 * P : (oc2 + 1) * P, nsl]
                        )
                        # out = (pp * R) + pb + x  (normalization folded here)
                        t1 = epil.tile([P, BW], f32, tag="t1", name="t1")
                        nc.vector.tensor_tensor(out=t1, in0=pp, in1=r_t,
                                                op=Alu.mult)
                        ot = epil.tile([P, BW], f32, tag="ot", name="ot")
                        nc.vector.scalar_tensor_tensor(
                            out=ot, in0=t1, scalar=pb_sb[oc2], in1=xt,
                            op0=Alu.add, op1=Alu.add,
                        )
                        nc.sync.dma_start(
                            out=out_d[oc2 * P : (oc2 + 1) * P, nsl], in_=ot
                        )

                pending = None
                for nb in range(NB):
                    nsl = slice(nb * BW, (nb + 1) * BW)
                    ps_s = s_ps.tile([P, BW], f32, tag="s", name="ps_s")
                    ph = [h_ps.tile([P, BW], f32, tag=f"h{oc}", name=f"hps{oc}")
                          for oc in range(NCH)]
                    if USE_FP8:
                        for pr in range(MPAIR):
                            et = expt.tile([P, 2, BW], fp8, tag="et", name="et")
                            for j in range(2):
                                mt = 2 * pr + j
                                msl = slice(mt * P, (mt + 1) * P)
                                pe = e_ps.tile([P, BW], f32, tag="e", name="pe")
                                for op in range(NOP):
                                    nc.tensor.matmul(
                                        out=pe, lhsT=k_sb[op][:, 0:2, msl],
                                        rhs=q_sb[op][:, 0:2, nsl],
                                        start=(op == 0), stop=(op == NOP - 1),
                                        perf_mode=DR,
                                    )
                                nc.scalar.activation(
                                    out=et[:, j, :], in_=pe, func=Act.Exp,
                                    bias=negc_sb, scale=1.0,
                                )
                            nc.tensor.matmul(
                                out=ps_s, lhsT=ones8_sb, rhs=et,
                                start=(pr == 0), stop=(pr == MPAIR - 1),
                                perf_mode=DR,
                            )
                            for oc in range(NCH):
                                nc.tensor.matmul(
                                    out=ph[oc],
                                    lhsT=vt_sb[pr][:, 0:2, oc * P : (oc + 1) * P],
                                    rhs=et,
                                    start=(pr == 0), stop=(pr == MPAIR - 1),
                                    perf_mode=DR,
                                )
                    else:
                        for mt in range(MT):
                            msl = slice(mt * P, (mt + 1) * P)
                            pe = e_ps.tile([P, BW], f32, tag="e", name="pe")
                            for oc in range(NCH):
                                nc.tensor.matmul(
                                    out=pe, lhsT=k_sb[oc][:, msl],
                                    rhs=q_sb[oc][:, nsl],
                                    start=(oc == 0), stop=(oc == NCH - 1),
                                )
                            et = expt.tile([P, BW], bf16, tag="et", name="et")
                            nc.scalar.activation(
                                out=et, in_=pe, func=Act.Exp,
                                bias=negc_sb, scale=1.0,
                            )
                            nc.tensor.matmul(
                                out=ps_s, lhsT=ones_sb, rhs=et,
                                start=(mt == 0), stop=(mt == MT - 1),
                            )
                            for oc in range(NCH):
                                nc.tensor.matmul(
                                    out=ph[oc],
                                    lhsT=vt_sb[mt][:, oc * P : (oc + 1) * P],
                                    rhs=et,
                                    start=(mt == 0), stop=(mt == MT - 1),
                                )
                    # epilogue: R = 1/S first (frees the S PSUM bank ASAP),
                    # then unnormalized h_attn -> bf16 split across ACT/DVE
                    r_t = epil.tile([P, BW], f32, tag="r", name="r_t")
                    nc.vector.reciprocal_approx_fast(out=r_t, in_=ps_s)
                    hu = []
                    for oc in range(NCH):
                        t = epil.tile([P, BW], bf16, tag=f"hu{oc}",
                                      name=f"hu{oc}")
                        if oc < 2:
                            nc.scalar.copy(out=t, in_=ph[oc])
                        else:
                            nc.vector.tensor_copy(out=t, in_=ph[oc])
                        hu.append(t)
                    if pending is not None:
                        emit_proj(*pending)
                    pending = (nb, hu, r_t)
                emit_proj(*pending)

    nc.compile()
    return nc


def _build_exec():
    import jax
    from jax.experimental.shard_map import shard_map
    from jax.sharding import Mesh, PartitionSpec

    from concourse import bass2jax, mybir

    nc = _build_nc()
    bass2jax.install_neuronx_cc_hook()

    partition_name = (
        nc.partition_id_tensor.name if nc.partition_id_tensor else None
    )
    in_names, out_names, out_avals = [], [], []
    for alloc in nc.m.functions[0].allocations:
        if not isinstance(alloc, mybir.MemoryLocationSet):
            continue
        name = alloc.memorylocations[0].name
        if alloc.kind == "ExternalInput":
            if name != partition_name:
                in_names.append(name)
        elif alloc.kind == "ExternalOutput":
            out_names.append(name)
            out_avals.append(
                jax.core.ShapedArray(
                    tuple(alloc.tensor_shape), mybir.dt.np(alloc.dtype)
                )
            )
    n_params = len(in_names)
    all_in = tuple(in_names + out_names)
    if partition_name is not None:
        all_in = all_in + (partition_name,)
    donate = tuple(range(n_params, n_params + len(out_names)))

    def _body(*args):
        operands = list(args)
        if partition_name is not None:
            operands.append(bass2jax.partition_id_tensor())
        outs = bass2jax._bass_exec_p.bind(
            *operands,
            out_avals=tuple(out_avals),
            in_names=all_in,
            out_names=tuple(out_names),
            lowering_input_output_aliases=(),
            sim_require_finite=True,
            sim_require_nnan=True,
            nc=nc,
        )
        return tuple(outs)

    devices = jax.devices()[:NCORES]
    mesh = Mesh(np.asarray(devices), ("core",))
    in_specs = (PartitionSpec("core"),) * (n_params + len(out_names))
    out_specs = (PartitionSpec("core"),) * len(out_names)
    sharded = jax.jit(
        shard_map(
            _body, mesh=mesh, in_specs=in_specs, out_specs=out_specs,
            check_rep=False,
        ),
        donate_argnums=donate,
        keep_unused=True,
    )
    return sharded, in_names, out_names, out_avals, nc


def _get_exec():
    global _EXEC
    if _EXEC is None:
        _EXEC = _build_exec()
    return _EXEC


def _selsum():
    s = np.zeros((P, GPC), np.float32)
    s[np.arange(P), np.arange(P) // GS] = 1.0
    return s


def make_concat_inputs(inputs):
    """Host-side prep: per-core shards concatenated on axis 0 (shard_map)."""
    x = np.asarray(inputs["x"], np.float32).reshape(B, C, N)
    scale = np.float32(C ** -0.5)

    def wT(w, s=None):
        w = np.asarray(w, np.float32)
        if s is not None:
            w = w * s
        return np.ascontiguousarray(w.T).astype(BF16)

    shared = {
        "wq": wT(inputs["q_w"], scale),
        "wk": wT(inputs["k_w"]),
        "wv": wT(inputs["v_w"]),
        "wp": wT(inputs["proj_w"]),
        "qb": (np.asarray(inputs["q_b"], np.float32) * scale).reshape(C, 1),
        "kb": np.asarray(inputs["k_b"], np.float32).reshape(C, 1),
        "pb": np.asarray(inputs["proj_b"], np.float32).reshape(C, 1),
        "vbb": np.ascontiguousarray(
            np.broadcast_to(
                np.asarray(inputs["v_b"], np.float32)[None, :], (P, BW)
            )
        ),
        "gamma": np.asarray(inputs["gamma"], np.float32).reshape(C, 1),
        "beta": np.asarray(inputs["beta"], np.float32).reshape(C, 1),
        "selsum": _selsum(),
        "selbc": np.ascontiguousarray(_selsum().T),
        "ones8": np.ones((P, 2 * P), FP8),
    }
    per_core = [dict(shared, x=np.ascontiguousarray(x[c]))
                for c in range(NCORES)]

    sharded, in_names, out_names, out_avals, _ = _get_exec()
    concat_in = [
        np.concatenate([per_core[c][nm] for c in range(NCORES)], axis=0)
        for nm in in_names
    ]
    return concat_in, out_avals


def run_concat(concat_in, out_avals):
    sharded = _get_exec()[0]
    concat_zeros = [
        np.zeros((NCORES * av.shape[0], *av.shape[1:]), av.dtype)
        for av in out_avals
    ]
    outs = sharded(*concat_in, *concat_zeros)
    return outs


def kernel(**inputs):
    concat_in, out_avals = make_concat_inputs(inputs)
    outs = run_concat(concat_in, out_avals)
    o = np.asarray(outs[0]).reshape(NCORES, C, N)
    return np.ascontiguousarray(o.reshape(B, C, H, W), dtype=np.float32)
